# revision 22
# baseline (speedup 1.0000x reference)
"""Trainium2 Bass kernel for a dense transformer block (pre-LN, MHA + MLP).

Sharding: 8 cores. Batch (B=2) is split across two 4-core groups; within a
group each core computes the full LayerNorm/K/V for its batch (2048 tokens)
but only its own 512 query rows through attention, proj, and the MLP.
No collectives: the host rolls each core's batch so its query rows are
tokens [0:512], and the host gathers the 8 x [512, 1024] results.

On-device dataflow (per core), all matmuls in bf16 with fp32 PSUM:
  LN1 (bn_stats/bn_aggr, rstd = 1/sqrt(var+eps)) -> xhat (bf16)
  PE-transpose xhat -> xhatT [C, tok]
  qT/kT = W^T @ xhat^T (transposed layout), v natural + ones column (vaug)
  S^T = kT_h^T @ qT_h per head (K=64), exp on ACT -> bf16
  yT_h (+Z row) = vaug^T @ expS, normalize by 1/Z via K=1 broadcast matmul
  proj natural via lhsT=yT (K=64 per head), +bias via K=1 ones-matmul,
  residual in fp32, LN2 -> xhat2T, MLP1 -> hT with fused exact-Gelu+bias,
  MLP2 natural via lhsT=hT, +bias, residual -> out (fp32)

gamma/beta of both LayerNorms and the q-scale are folded into the weight
matrices host-side (LN(x) @ W == xhat @ (gamma*W) + beta@W).

SBUF pressure is handled by tag-sharing slots between tensors with disjoint
lifetimes: xhatT<->hT, kT<->xr, vaug<->x2, qT<->xhat2T, and the wq/wk/wv
group-half slots are recycled by the W2 half tiles.
"""

import numpy as np
import ml_dtypes

import concourse.bass as bass
import concourse.tile as tile
from concourse import bacc, mybir
from concourse.bass_utils import run_bass_kernel_spmd
from concourse.masks import make_identity

F32 = mybir.dt.float32
BF16 = mybir.dt.bfloat16
AF = mybir.ActivationFunctionType
ALU = mybir.AluOpType

B, N, C, F, H, D = 2, 2048, 1024, 4096, 16, 64
QR = 512            # query rows per core
CH = C // 128       # 8 C-chunks
FT = F // 128       # 32 F-tiles
NT = N // 128       # 16 token tiles
EPS = 1e-6
NCORES = 8

_NC_CACHE = {}


def _ln_rstd(nc, stat, mv, eps_sb):
    """rstd = 1/sqrt(var + eps): ACT Sqrt (batched per phase, one table
    load) + DVE reciprocal (groupnorm-style)."""
    sd = stat.tile([128, 1], F32, tag="sd", name="sd")
    nc.scalar.activation(out=sd, in_=mv[:, 1:2], func=AF.Sqrt, bias=eps_sb, scale=1.0)
    rstd = stat.tile([128, 1], F32, tag="rstd", name="rstd")
    nc.vector.reciprocal(out=rstd, in_=sd)
    return rstd


def _build_nc(zero_bias_r=False):
    nc = bacc.Bacc("TRN2", target_bir_lowering=False, debug=False,
                   enable_asserts=False, num_devices=NCORES)

    xb = nc.dram_tensor("xb", [N, C], BF16, kind="ExternalInput")
    xr = nc.dram_tensor("xr", [QR, C], F32, kind="ExternalInput")
    wq = nc.dram_tensor("wq", [C, C], BF16, kind="ExternalInput")
    wk = nc.dram_tensor("wk", [C, C], BF16, kind="ExternalInput")
    wv = nc.dram_tensor("wv", [C, C], BF16, kind="ExternalInput")
    wp = nc.dram_tensor("wp", [C, C], BF16, kind="ExternalInput")
    w1t = nc.dram_tensor("w1t", [FT, 128, CH, 128], BF16, kind="ExternalInput")
    w2 = nc.dram_tensor("w2", [F, C], BF16, kind="ExternalInput")
    biasT = nc.dram_tensor("biasT", [128, CH + CH + FT], F32, kind="ExternalInput")
    biasR = nc.dram_tensor("biasR", [3, C], BF16, kind="ExternalInput")
    out = nc.dram_tensor("out", [QR, C], F32, kind="ExternalOutput")

    with tile.TileContext(nc) as tc, \
         tc.tile_pool(name="consts", bufs=1) as consts, \
         tc.tile_pool(name="wc", bufs=4) as wcp, \
         tc.tile_pool(name="w1p", bufs=3) as w1p, \
         tc.tile_pool(name="big", bufs=1) as big, \
         tc.tile_pool(name="xin", bufs=2) as xin, \
         tc.tile_pool(name="xhp", bufs=3) as xhp, \
         tc.tile_pool(name="stat", bufs=4) as stat, \
         tc.tile_pool(name="expp", bufs=3) as expp, \
         tc.tile_pool(name="outp", bufs=1) as outp, \
         tc.tile_pool(name="zp", bufs=1) as zp, \
         tc.tile_pool(name="ps", bufs=2, space="PSUM") as ps:

        # ---- constants ----
        ident = consts.tile([128, 128], BF16, name="ident")
        make_identity(nc, ident)
        onesP = consts.tile([128, 128], BF16, name="onesP")
        nc.vector.memset(onesP, 1.0)
        ones1 = onesP[0:1, :]
        eps_sb = consts.tile([128, 1], F32, name="eps_sb")
        nc.vector.memset(eps_sb, EPS)
        biasT_sb = consts.tile([128, CH + CH + FT], F32, name="biasT_sb")
        nc.sync.dma_start(out=biasT_sb, in_=biasT.ap())
        bq_sb = biasT_sb[:, 0:CH]
        bk_sb = biasT_sb[:, CH:2 * CH]
        b1_sb = biasT_sb[:, 2 * CH:2 * CH + FT]
        # bv/bp/b2 rows live on partitions 0/32/64 of one tile so each can be
        # the rhs of a K=1 ones-matmul (rhs base partition must be 0/32/64).
        if not zero_bias_r:
            biasRow = consts.tile([65, C], BF16, name="biasRow")
            nc.sync.dma_start(out=biasRow[::32, :], in_=biasR.ap())
            bv_sb = biasRow[0:1, :]
            bp_sb = biasRow[32:33, :]
            b2_sb = biasRow[64:65, :]
        else:
            bv_sb = bp_sb = b2_sb = None

        # ---- LN1 + transpose -> xhatT [128, CH, N] (bf16) ----
        xhatT = big.tile([128, CH, N], BF16, tag="xhatT_hT", name="xhatT")
        for t2 in range(NT // 2):
            xt2 = xin.tile([128, 2, C], BF16, tag="xt", name="xt2")
            nc.sync.dma_start(
                out=xt2, in_=xb.ap()[t2 * 256:(t2 + 1) * 256, :]
                .rearrange("(i p) n -> p i n", p=128))
            for i in range(2):
                t = t2 * 2 + i
                xt = xt2[:, i, :]
                st = stat.tile([128, 2, 6], F32, tag="bns", name="st")
                nc.vector.bn_stats(out=st[:, 0, :], in_=xt[:, 0:512])
                nc.vector.bn_stats(out=st[:, 1, :], in_=xt[:, 512:1024])
                mv = stat.tile([128, 2], F32, tag="mv", name="mv")
                nc.vector.bn_aggr(out=mv, in_=st)
                rstd = _ln_rstd(nc, stat, mv, eps_sb)
                xh = xhp.tile([128, C], BF16, tag="xhat", name="xh", bufs=2)
                nc.vector.tensor_scalar(out=xh, in0=xt, scalar1=mv[:, 0:1],
                                        scalar2=rstd,
                                        op0=ALU.subtract, op1=ALU.mult)
                for j in range(CH):
                    pst = ps.tile([128, 128], BF16, tag="mm512", name="pst")
                    nc.tensor.transpose(pst, xh[:, j * 128:(j + 1) * 128], ident)
                    nc.vector.tensor_copy(out=xhatT[:, j, t * 128:(t + 1) * 128],
                                          in_=pst)

        # ---- QKV + attention, two 8-head groups ----
        # yT uses a 64-partition layout (one head per chunk) so every DVE op
        # stays partition-aligned (DVE cannot move data across partitions).
        # The attention phase is ACT(exp)-bound, so PE filler work is
        # interleaved into it: group-1 kT matmuls fill group-0's attention,
        # and the first half of the proj accumulation fills group-1's.
        yT = big.tile([64, H, QR], BF16, name="yT")

        def dma_w(w, g, name):
            wt = wcp.tile([128, CH, 512], BF16, tag="wh", name=name)
            nc.sync.dma_start(
                out=wt, in_=w.ap()[:, g * 512:(g + 1) * 512]
                .rearrange("(c p) n -> p c n", p=128))
            return wt

        def emit_qT(g, wq_h, qT_g):
            for j in range(4):
                psq = ps.tile([128, 512], F32, tag="mm512", name="psq")
                for c in range(CH):
                    nc.tensor.matmul(psq, lhsT=wq_h[:, c, j * 128:(j + 1) * 128],
                                     rhs=xhatT[:, c, 0:QR],
                                     start=(c == 0), stop=(c == CH - 1))
                nc.vector.tensor_scalar_add(out=qT_g[:, j, :], in0=psq,
                                            scalar1=bq_sb[:, 4 * g + j:4 * g + j + 1])

        def emit_kT_chunk(g, wk_h, kT_g, j, sl):
            psk = ps.tile([128, 512], F32, tag="mm512", name="psk")
            for c in range(CH):
                nc.tensor.matmul(psk, lhsT=wk_h[:, c, j * 128:(j + 1) * 128],
                                 rhs=xhatT[:, c, sl * 512:(sl + 1) * 512],
                                 start=(c == 0), stop=(c == CH - 1))
            nc.vector.tensor_scalar_add(
                out=kT_g[:, j, sl * 512:(sl + 1) * 512], in0=psk,
                scalar1=bk_sb[:, 4 * g + j:4 * g + j + 1])

        def emit_v(g, wv_h, vaug):
            nc.vector.memset(vaug[:, :, :, 64:65], 1.0)
            for tt in range(NT):
                psv = ps.tile([128, 512], F32, tag="mm512", name="psv")
                for c in range(CH):
                    nc.tensor.matmul(psv, lhsT=xhatT[:, c, tt * 128:(tt + 1) * 128],
                                     rhs=wv_h[:, c, :], start=(c == 0),
                                     stop=(zero_bias_r and c == CH - 1))
                if not zero_bias_r:
                    nc.tensor.matmul(psv, lhsT=ones1,
                                     rhs=bv_sb[:, g * 512:(g + 1) * 512],
                                     start=False, stop=True)
                nc.vector.tensor_copy(out=vaug[:, tt, :, 0:64],
                                      in_=psv.rearrange("p (h d) -> p h d", h=8))

        def emit_attn_head(g, hh, kT_g, qT_g, vaug, filler):
            jj = hh // 2
            poff = (hh % 2) * 64
            psY = ps.tile([65, 512], F32, tag="y", name="psY")
            # paired tp-blocks: issue both S matmuls + exps of the pair before
            # any y matmul, so ACT (the attention bottleneck) is never starved
            # behind a y that waits on an earlier exp.
            for pb in range(4):
                exps = []
                for u in range(2):
                    tp = 2 * pb + u
                    psS = ps.tile([128, 1024], F32, tag="s1024", name="psS")
                    for half in range(2):
                        tt = 2 * tp + half
                        nc.tensor.matmul(
                            psS[:, half * 512:(half + 1) * 512],
                            lhsT=kT_g[poff:poff + 64, jj, tt * 128:(tt + 1) * 128],
                            rhs=qT_g[poff:poff + 64, jj, :],
                            start=True, stop=True)
                    expS = expp.tile([128, 1024], BF16, tag="expS", name="expS")
                    nc.scalar.activation(out=expS, in_=psS, func=AF.Exp)
                    exps.append(expS)
                if pb == 0 and filler is not None:
                    filler()
                for u in range(2):
                    tp = 2 * pb + u
                    for half in range(2):
                        tt = 2 * tp + half
                        nc.tensor.matmul(
                            psY, lhsT=vaug[:, tt, hh, :],
                            rhs=exps[u][:, half * 512:(half + 1) * 512],
                            start=(tt == 0), stop=(tt == NT - 1))
            # softmax denominator: Z sits in psY row 64 (partition 64).
            # All DVE ops stay on partition 64; the K=1 broadcast matmul
            # (PE can cross partitions) fans 1/Z out to partitions 0..63.
            rzb = zp.tile([65, 512], BF16, tag="rzb", name="rzb")
            with nc.allow_low_precision("1/Z feeds a bf16 broadcast matmul"):
                nc.vector.reciprocal(out=rzb[64:65, :], in_=psY[64:65, :])
            psZB = ps.tile([64, 512], F32, tag="mm512", name="psZB")
            nc.tensor.matmul(psZB, lhsT=onesP[64:65, 0:64], rhs=rzb[64:65, :],
                             start=True, stop=True)
            zb_sb = zp.tile([64, 512], F32, tag="zb", name="zb_sb")
            nc.vector.tensor_copy(out=zb_sb, in_=psZB)
            nc.vector.tensor_mul(out=yT[:, 8 * g + hh, :],
                                 in0=psY[0:64, :], in1=zb_sb)

        # group 0 QKV
        wq_h0 = dma_w(wq, 0, "wq_h0")
        wk_h0 = dma_w(wk, 0, "wk_h0")
        wv_h0 = dma_w(wv, 0, "wv_h0")
        qT_g0 = big.tile([128, 4, QR], BF16, tag="qT_xh2T", name="qT_g0")
        emit_qT(0, wq_h0, qT_g0)
        kT_g0 = big.tile([128, 4, N], BF16, tag="kT_xr", name="kT_g0", bufs=2)
        for j in range(4):
            for sl in range(4):
                emit_kT_chunk(0, wk_h0, kT_g0, j, sl)
        vaug0 = big.tile([128, NT, 8, 65], BF16, tag="vaug_x2", name="vaug0")
        emit_v(0, wv_h0, vaug0)

        # group-0 attention with group-1 kT as PE filler
        wk_h1 = dma_w(wk, 1, "wk_h1")
        kT_g1 = big.tile([128, 4, N], BF16, tag="kT_xr", name="kT_g1", bufs=2)

        def kt1_filler(hh):
            def fill():
                for u in range(2):
                    idx = 2 * hh + u
                    emit_kT_chunk(1, wk_h1, kT_g1, idx // 4, idx % 4)
            return fill

        for hh in range(8):
            emit_attn_head(0, hh, kT_g0, qT_g0, vaug0, kt1_filler(hh))

        # group 1 remaining QKV
        wq_h1 = dma_w(wq, 1, "wq_h1")
        wv_h1 = dma_w(wv, 1, "wv_h1")
        qT_g1 = big.tile([128, 4, QR], BF16, tag="qT_xh2T", name="qT_g1")
        emit_qT(1, wq_h1, qT_g1)
        vaug1 = big.tile([128, NT, 8, 65], BF16, tag="vaug_x2", name="vaug1")
        emit_v(1, wv_h1, vaug1)

        # group-1 attention with first-half proj (heads 0..7) as PE filler;
        # partial sums spill to SBUF in bf16 (tiny vs the fp32 residual).
        wp_t0 = wcp.tile([64, H, 512], BF16, tag="wp", name="wp_t0", bufs=2)
        nc.sync.dma_start(out=wp_t0, in_=wp.ap()[:, 0:512]
                          .rearrange("(h p) n -> p h n", p=64))
        wp_t1 = wcp.tile([64, H, 512], BF16, tag="wp", name="wp_t1", bufs=2)
        nc.sync.dma_start(out=wp_t1, in_=wp.ap()[:, 512:1024]
                          .rearrange("(h p) n -> p h n", p=64))
        wp_ts = [wp_t0, wp_t1]
        proj_part = xhp.tile([128, QR // 128, C], BF16, tag="ppart", name="proj_part",
                             bufs=1)

        def proj_filler(hh):
            os_, qt = hh // 4, hh % 4

            def fill():
                psp = ps.tile([128, 512], F32, tag="mm512", name="psp")
                for h in range(8):
                    nc.tensor.matmul(psp, lhsT=yT[:, h, qt * 128:(qt + 1) * 128],
                                     rhs=wp_ts[os_][:, h, :],
                                     start=(h == 0), stop=(h == 7))
                nc.vector.tensor_copy(
                    out=proj_part[:, qt, os_ * 512:(os_ + 1) * 512], in_=psp)
            return fill

        for hh in range(8):
            emit_attn_head(1, hh, kT_g1, qT_g1, vaug1, proj_filler(hh))

        # ---- proj second half (heads 8..15) + partial + residual -> x2 ----
        xr_sb = big.tile([128, QR // 128, C], F32, tag="kT_xr", name="xr_sb", bufs=2)
        nc.sync.dma_start(out=xr_sb, in_=xr.ap().rearrange("(q p) c -> p q c", p=128))
        x2 = big.tile([128, QR // 128, C], F32, tag="vaug_x2", name="x2")
        for os_ in range(2):
            for qt in range(QR // 128):
                psp = ps.tile([128, 512], F32, tag="mm512", name="psp2")
                for h in range(8, H):
                    nc.tensor.matmul(psp, lhsT=yT[:, h, qt * 128:(qt + 1) * 128],
                                     rhs=wp_ts[os_][:, h, :], start=(h == 8),
                                     stop=(zero_bias_r and h == H - 1))
                if not zero_bias_r:
                    nc.tensor.matmul(psp, lhsT=onesP[32:33, :],
                                     rhs=bp_sb[:, os_ * 512:(os_ + 1) * 512],
                                     start=False, stop=True)
                sl = slice(os_ * 512, (os_ + 1) * 512)
                nc.vector.tensor_add(out=x2[:, qt, sl], in0=psp,
                                     in1=proj_part[:, qt, sl])
                nc.vector.tensor_add(out=x2[:, qt, sl], in0=x2[:, qt, sl],
                                     in1=xr_sb[:, qt, sl])

        # ---- LN2 + transpose -> xhat2T [128, CH, QR] ----
        xh2T = big.tile([128, CH, QR], BF16, tag="qT_xh2T", name="xh2T")
        for qt in range(QR // 128):
            st2 = stat.tile([128, 2, 6], F32, tag="bns", name="st2")
            nc.vector.bn_stats(out=st2[:, 0, :], in_=x2[:, qt, 0:512])
            nc.vector.bn_stats(out=st2[:, 1, :], in_=x2[:, qt, 512:1024])
            mv2 = stat.tile([128, 2], F32, tag="mv", name="mv2")
            nc.vector.bn_aggr(out=mv2, in_=st2)
            rstd2 = _ln_rstd(nc, stat, mv2, eps_sb)
            xh2 = xhp.tile([128, C], BF16, tag="xhat", name="xh2", bufs=2)
            nc.vector.tensor_scalar(out=xh2, in0=x2[:, qt, :], scalar1=mv2[:, 0:1],
                                    scalar2=rstd2, op0=ALU.subtract, op1=ALU.mult)
            for j in range(CH):
                pst2 = ps.tile([128, 128], BF16, tag="mm512", name="pst2")
                nc.tensor.transpose(pst2, xh2[:, j * 128:(j + 1) * 128], ident)
                nc.vector.tensor_copy(out=xh2T[:, j, qt * 128:(qt + 1) * 128],
                                      in_=pst2)

        # ---- MLP1: hT [128, FT, QR] with fused exact Gelu + bias ----
        hT = big.tile([128, FT, QR], BF16, tag="xhatT_hT", name="hT")
        for ft in range(FT):
            w1tile = w1p.tile([128, CH, 128], BF16, tag="w1", name="w1tile")
            nc.sync.dma_start(out=w1tile, in_=w1t.ap()[ft])
            psh = ps.tile([128, 512], F32, tag="mm512", name="psh")
            for c in range(CH):
                nc.tensor.matmul(psh, lhsT=w1tile[:, c, :], rhs=xh2T[:, c, :],
                                 start=(c == 0), stop=(c == CH - 1))
            nc.scalar.activation(out=hT[:, ft, :], in_=psh, func=AF.Gelu,
                                 bias=b1_sb[:, ft:ft + 1], scale=1.0)

        # ---- MLP2 + residual -> out ----
        for os_ in range(2):
            w2_t = []
            for i in range(4):
                w2t = wcp.tile([128, CH, 512], BF16, tag="wh", name="w2t")
                nc.sync.dma_start(
                    out=w2t, in_=w2.ap()[i * 1024:(i + 1) * 1024,
                                         os_ * 512:(os_ + 1) * 512]
                    .rearrange("(c p) n -> p c n", p=128))
                w2_t.append(w2t)
            for qt in range(QR // 128):
                pso = ps.tile([128, 512], F32, tag="mm512", name="pso")
                for fc in range(FT):
                    nc.tensor.matmul(
                        pso, lhsT=hT[:, fc, qt * 128:(qt + 1) * 128],
                        rhs=w2_t[fc // 8][:, fc % 8, :],
                        start=(fc == 0), stop=(zero_bias_r and fc == FT - 1))
                if not zero_bias_r:
                    nc.tensor.matmul(pso, lhsT=onesP[64:65, :],
                                     rhs=b2_sb[:, os_ * 512:(os_ + 1) * 512],
                                     start=False, stop=True)
                ot = outp.tile([128, 512], F32, tag="ot", name="ot")
                nc.vector.tensor_add(out=ot, in0=pso,
                                     in1=x2[:, qt, os_ * 512:(os_ + 1) * 512])
                nc.sync.dma_start(out=out.ap()[qt * 128:(qt + 1) * 128,
                                               os_ * 512:(os_ + 1) * 512], in_=ot)

    nc.finalize()
    return nc


def _get_nc(zero_bias_r=False):
    key = ("nc", zero_bias_r)
    if key not in _NC_CACHE:
        _NC_CACHE[key] = _build_nc(zero_bias_r)
    return _NC_CACHE[key]


def kernel(x, Wq, Wk, Wv, Wp, bp, W1, b1, W2, b2, gamma1, beta1, gamma2, beta2):
    bf = ml_dtypes.bfloat16
    x = np.asarray(x, np.float32)
    Wq = np.asarray(Wq, np.float32)
    Wk = np.asarray(Wk, np.float32)
    Wv = np.asarray(Wv, np.float32)
    Wp = np.asarray(Wp, np.float32)
    bp = np.asarray(bp, np.float32)
    W1 = np.asarray(W1, np.float32)
    b1 = np.asarray(b1, np.float32)
    W2 = np.asarray(W2, np.float32)
    b2 = np.asarray(b2, np.float32)
    gamma1 = np.asarray(gamma1, np.float32)
    beta1 = np.asarray(beta1, np.float32)
    gamma2 = np.asarray(gamma2, np.float32)
    beta2 = np.asarray(beta2, np.float32)

    scale = np.float32(D ** -0.5)
    wq_f = ((gamma1[:, None] * Wq) * scale).astype(bf)
    bq_f = (beta1 @ Wq) * scale
    wk_f = (gamma1[:, None] * Wk).astype(bf)
    bk_f = beta1 @ Wk
    wv_f = (gamma1[:, None] * Wv).astype(bf)
    bv_f = beta1 @ Wv
    w1_f = gamma2[:, None] * W1
    b1_f = b1 + beta2 @ W1
    w1_tiled = np.ascontiguousarray(
        w1_f.reshape(CH, 128, FT, 128).transpose(2, 1, 0, 3)).astype(bf)
    biasT = np.ascontiguousarray(np.concatenate(
        [bq_f.reshape(CH, 128).T, bk_f.reshape(CH, 128).T,
         b1_f.reshape(FT, 128).T], axis=1).astype(np.float32))
    biasR = np.stack([bv_f, bp, b2]).astype(bf)

    common = {
        "wq": wq_f, "wk": wk_f, "wv": wv_f, "wp": Wp.astype(bf),
        "w1t": w1_tiled, "w2": W2.astype(bf),
        "biasT": biasT, "biasR": biasR,
    }

    in_maps = []
    for core in range(NCORES):
        b = core // 4
        qoff = (core % 4) * QR
        xroll = np.roll(x[b], -qoff, axis=0)
        m = dict(common)
        m["xb"] = xroll.astype(bf)
        m["xr"] = np.ascontiguousarray(x[b][qoff:qoff + QR])
        in_maps.append(m)

    zero_bias_r = not (np.any(bv_f) or np.any(bp) or np.any(b2))
    nc = _get_nc(zero_bias_r)
    _NC_CACHE["last_nc"] = nc
    res = run_bass_kernel_spmd(nc, in_maps, core_ids=list(range(NCORES)))
    _NC_CACHE["last_result"] = res

    outp = np.empty((B, N, C), np.float32)
    for core in range(NCORES):
        b = core // 4
        qoff = (core % 4) * QR
        outp[b, qoff:qoff + QR] = res.results[core]["out"]
    return outp
